# revision 7
# baseline (speedup 1.0000x reference)
import sys

if "/opt/trn_rl_repo" not in sys.path:
    sys.path.insert(0, "/opt/trn_rl_repo")

from contextlib import ExitStack

import numpy as np

import concourse.bacc as bacc
import concourse.bass as bass
import concourse.mybir as mybir
import concourse.tile as tile
from concourse.bass_utils import run_bass_kernel_spmd

B, H, N, T, D = 4, 4, 32, 96, 32
DQK = T * D  # 3072
SCALE = float(DQK**0.5)
NCORES = 8
NCH = DQK // 128  # 24 contraction chunks for Q.K
KT = 8  # V row tiles per (b,h): 1024 rows / 128
NB = DQK // 512  # 6 psum column chunks
F32 = mybir.dt.float32
F32R = mybir.dt.float32r
NEG = -1.0e30


def _build_program():
    nc = bacc.Bacc()
    qkt_d = nc.declare_dram_parameter("qkt", [128, NCH * 128], F32, isOutput=False)
    mb_d = nc.declare_dram_parameter("mb", [32, 64], F32, isOutput=False)
    v_d = nc.declare_dram_parameter("v", [2, KT * 128, DQK], F32R, isOutput=False)
    mc_d = nc.declare_dram_parameter("mconst", [128, KT * 32], F32, isOutput=False)
    i4_d = nc.declare_dram_parameter("i4t", [32, 128], F32, isOutput=False)
    out_d = nc.declare_dram_parameter("out", [2, 32, DQK], F32, isOutput=True)

    with tile.TileContext(nc) as tc, ExitStack() as ctx:
        sb = ctx.enter_context(tc.tile_pool(name="sb", bufs=1))
        vp = ctx.enter_context(tc.tile_pool(name="vp", bufs=4))
        outp = ctx.enter_context(tc.tile_pool(name="outp", bufs=2))
        pg = ctx.enter_context(tc.tile_pool(name="pg", bufs=1, space="PSUM"))
        pr = ctx.enter_context(tc.tile_pool(name="pr", bufs=1, space="PSUM"))
        po = ctx.enter_context(tc.tile_pool(name="po", bufs=1, space="PSUM"))

        qkt_sb = sb.tile([128, NCH * 128], F32, tag="qkt")
        mb_sb = sb.tile([32, 64], F32, tag="mb")
        mc_sb = sb.tile([128, KT * 32], F32, tag="mc")
        i4_sb = sb.tile([32, 128], F32, tag="i4")
        a2_sb = sb.tile([128, 2 * KT * 32], F32R, tag="a2")
        t_sb = sb.tile([32, 64], F32, tag="t")
        e_sb = sb.tile([32, 64], F32, tag="e")
        eT_sb = sb.tile([32, 64], F32, tag="eT")
        nm_sb = sb.tile([32, 2], F32, tag="nm")
        nms_sb = sb.tile([32, 2], F32, tag="nms")
        rs_sb = sb.tile([32, 2], F32, tag="rs")
        ri_sb = sb.tile([32, 2], F32, tag="ri")

        nc.gpsimd.dma_start(mc_sb[:, :], mc_d[:, :])
        nc.gpsimd.dma_start(i4_sb[:, :], i4_d[:, :])
        nc.gpsimd.dma_start(mb_sb[:, :], mb_d[:, :])
        nc.gpsimd.dma_start(qkt_sb[:, :], qkt_d[:, :])

        # Gram matrix of the stacked [Q0 Q1 K0 K1] columns: one [128,128]
        # PSUM accumulator over 24 contraction chunks of 128.
        gram = pg.tile([128, 128], F32, tag="gram")
        for c in range(NCH):
            sl = qkt_sb[:, 128 * c : 128 * (c + 1)]
            nc.tensor.matmul(
                gram[:, :], sl, sl, start=(c == 0), stop=(c == NCH - 1)
            )

        for bh in range(2):
            blk = gram[32 * bh : 32 * bh + 32, 64 + 32 * bh : 96 + 32 * bh]
            tcur = t_sb[:, 32 * bh : 32 * bh + 32]
            nc.vector.tensor_tensor(
                tcur, blk, mb_sb[:, 32 * bh : 32 * bh + 32], mybir.AluOpType.add
            )
            nm = nm_sb[:, bh : bh + 1]
            nc.vector.reduce_max(nm, tcur, axis=mybir.AxisListType.X, negate=True)
            nms = nms_sb[:, bh : bh + 1]
            nc.vector.tensor_scalar_mul(nms, nm, 1.0 / SCALE)
            ecur = e_sb[:, 32 * bh : 32 * bh + 32]
            rs = rs_sb[:, bh : bh + 1]
            nc.scalar.activation(
                ecur,
                tcur,
                mybir.ActivationFunctionType.Exp,
                bias=nms,
                scale=1.0 / SCALE,
                accum_out=rs,
            )
            nc.vector.reciprocal(ri_sb[:, bh : bh + 1], rs)
            eT = eT_sb[:, 32 * bh : 32 * bh + 32]
            nc.vector.transpose(eT, ecur)
            rep = pr.tile([128, 32], F32, tag="rep")
            nc.tensor.matmul(rep[:, :], i4_sb[:, :], eT, start=True, stop=True)
            for kt in range(KT):
                c0 = 32 * (KT * bh + kt)
                nc.vector.tensor_tensor(
                    a2_sb[:, c0 : c0 + 32],
                    rep[:, :],
                    mc_sb[:, 32 * kt : 32 * kt + 32],
                    mybir.AluOpType.mult,
                )

        for bh in range(2):
            ops_t = po.tile([32, DQK], F32, tag="out")
            for kt in range(KT):
                vt = vp.tile([128, DQK], F32R, tag="v")
                nc.sync.dma_start(vt[:, :], v_d[bh, 128 * kt : 128 * (kt + 1), :])
                c0 = 32 * (KT * bh + kt)
                a2c = a2_sb[:, c0 : c0 + 32]
                for n in range(NB):
                    nc.tensor.matmul(
                        ops_t[:, 512 * n : 512 * (n + 1)],
                        a2c,
                        vt[:, 512 * n : 512 * (n + 1)],
                        start=(kt == 0),
                        stop=(kt == KT - 1),
                    )
            ot = outp.tile([32, DQK], F32, tag="ot")
            for n in range(NB):
                nc.scalar.mul(
                    ot[:, 512 * n : 512 * (n + 1)],
                    ops_t[:, 512 * n : 512 * (n + 1)],
                    ri_sb[:, bh : bh + 1],
                )
            nc.sync.dma_start(out_d[bh], ot[:, :])

    nc.finalize()
    return nc


_PROG = None


def _get_program():
    global _PROG
    if _PROG is None:
        _PROG = _build_program()
    return _PROG


def _consts():
    mc = np.zeros((128, KT * 32), np.float32)
    for p in range(128):
        ii = p // 32
        for kt in range(KT):
            mc[p, 32 * kt + 4 * kt + ii] = 1.0
    i4t = np.tile(np.eye(32, dtype=np.float32), (1, 4))
    return mc, i4t


def make_in_maps(Q, K, V, mask):
    Q = np.asarray(Q)
    K = np.asarray(K)
    V = np.asarray(V)
    mask = np.asarray(mask)
    mc, i4t = _consts()
    in_maps = []
    for c in range(NCORES):
        pairs = [(2 * c) // H, (2 * c) % H], [(2 * c + 1) // H, (2 * c + 1) % H]
        cols = [Q[b, h].T for b, h in pairs] + [K[b, h].T for b, h in pairs]
        stack = np.concatenate(cols, axis=1)  # [3072, 128]
        qkt = np.ascontiguousarray(
            stack.reshape(NCH, 128, 128).transpose(1, 0, 2)
        ).reshape(128, NCH * 128)
        mb = np.concatenate(
            [
                np.where(mask[b, h] == 0, np.float32(NEG), np.float32(0.0))
                for b, h in pairs
            ],
            axis=1,
        ).astype(np.float32)
        v2 = np.stack(
            [
                np.ascontiguousarray(V[b, h].transpose(1, 0, 2, 3)).reshape(
                    KT * 128, DQK
                )
                for b, h in pairs
            ]
        )
        in_maps.append({"qkt": qkt, "mb": mb, "v": v2, "mconst": mc, "i4t": i4t})
    return in_maps


def kernel(Q=None, K=None, V=None, mask=None, _trace=False, **_ignored):
    in_maps = make_in_maps(Q, K, V, mask)
    nc = _get_program()
    res = run_bass_kernel_spmd(nc, in_maps, list(range(NCORES)), trace=_trace)
    outs = np.stack([r["out"] for r in res.results])  # [8, 2, 32, 3072]
    out = outs.reshape(B, H, N, T, D)
    if _trace:
        return out, res
    return out


# revision 17
# speedup vs baseline: 1.7467x; 1.7467x over previous
import sys

if "/opt/trn_rl_repo" not in sys.path:
    sys.path.insert(0, "/opt/trn_rl_repo")

from contextlib import ExitStack

import ml_dtypes
import numpy as np

import concourse.bacc as bacc
import concourse.bass as bass
import concourse.mybir as mybir
import concourse.tile as tile
from concourse.bass_utils import run_bass_kernel_spmd

B, H, N, T, D = 4, 4, 32, 96, 32
DQK = T * D  # 3072
SCALE = float(DQK**0.5)
NCORES = 8
NCH = DQK // 128  # 24 contraction chunks for Q.K
KT = 8  # V row tiles per (b,h): 1024 rows / 128
NB = DQK // 512  # 6 psum column chunks
F32 = mybir.dt.float32
F32R = mybir.dt.float32r
BF16 = mybir.dt.bfloat16
NEG = -1.0e30


def _build_program():
    nc = bacc.Bacc()
    qkt_d = nc.declare_dram_parameter("qkt", [128, NCH * 128], BF16, isOutput=False)
    mb_d = nc.declare_dram_parameter("mb", [32, 64], F32, isOutput=False)
    v_d = nc.declare_dram_parameter("v", [2, KT * 128, DQK], BF16, isOutput=False)
    mc_d = nc.declare_dram_parameter("mconst", [128, KT * 32], F32, isOutput=False)
    i4_d = nc.declare_dram_parameter("i4t", [32, 128], F32, isOutput=False)
    out_d = nc.declare_dram_parameter("out", [2, 32, DQK], F32, isOutput=True)

    with tile.TileContext(nc) as tc, ExitStack() as ctx:
        sb = ctx.enter_context(tc.tile_pool(name="sb", bufs=1))
        vp = ctx.enter_context(tc.tile_pool(name="vp", bufs=16))
        outp = ctx.enter_context(tc.tile_pool(name="outp", bufs=2))
        pg = ctx.enter_context(tc.tile_pool(name="pg", bufs=1, space="PSUM"))
        pr = ctx.enter_context(tc.tile_pool(name="pr", bufs=1, space="PSUM"))
        po = ctx.enter_context(tc.tile_pool(name="po", bufs=1, space="PSUM"))

        qkt_sb = sb.tile([128, NCH * 128], BF16, tag="qkt")
        mb_sb = sb.tile([32, 64], F32, tag="mb")
        mc_sb = sb.tile([128, KT * 32], F32, tag="mc")
        i4_sb = sb.tile([32, 128], F32, tag="i4")
        a2_sb = sb.tile([128, 2 * KT * 32], BF16, tag="a2")
        t_sb = sb.tile([32, 64], F32, tag="t")
        e_sb = sb.tile([32, 64], F32, tag="e")
        eT_sb = sb.tile([32, 64], F32, tag="eT")
        nm_sb = sb.tile([32, 2], F32, tag="nm")
        nms_sb = sb.tile([32, 2], F32, tag="nms")
        rs_sb = sb.tile([32, 2], F32, tag="rs")
        ri_sb = sb.tile([32, 2], F32, tag="ri")

        nc.scalar.dma_start(qkt_sb[:, :], qkt_d[:, :])
        nc.gpsimd.dma_start(mc_sb[:, :], mc_d[:, :])
        nc.gpsimd.dma_start(i4_sb[:, :], i4_d[:, :])
        nc.gpsimd.dma_start(mb_sb[:, :], mb_d[:, :])

        # Gram matrix of the stacked [Q0 Q1 K0 K1] columns: one [128,128]
        # PSUM accumulator over 24 contraction chunks of 128.
        gram = pg.tile([128, 128], F32, tag="gram")
        for c in range(NCH):
            sl = qkt_sb[:, 128 * c : 128 * (c + 1)]
            nc.tensor.matmul(
                gram[:, :], sl, sl, start=(c == 0), stop=(c == NCH - 1)
            )

        for bh in range(2):
            blk = gram[32 * bh : 32 * bh + 32, 64 + 32 * bh : 96 + 32 * bh]
            tcur = t_sb[:, 32 * bh : 32 * bh + 32]
            nc.vector.tensor_tensor(
                tcur, blk, mb_sb[:, 32 * bh : 32 * bh + 32], mybir.AluOpType.add
            )
            nm = nm_sb[:, bh : bh + 1]
            nc.vector.reduce_max(nm, tcur, axis=mybir.AxisListType.X, negate=True)
            nms = nms_sb[:, bh : bh + 1]
            nc.vector.tensor_scalar_mul(nms, nm, 1.0 / SCALE)
            ecur = e_sb[:, 32 * bh : 32 * bh + 32]
            rs = rs_sb[:, bh : bh + 1]
            nc.scalar.activation(
                ecur,
                tcur,
                mybir.ActivationFunctionType.Exp,
                bias=nms,
                scale=1.0 / SCALE,
                accum_out=rs,
            )
            nc.vector.reciprocal(ri_sb[:, bh : bh + 1], rs)
            eT = eT_sb[:, 32 * bh : 32 * bh + 32]
            nc.vector.transpose(eT, ecur)
            rep = pr.tile([128, 32], F32, tag="rep")
            nc.tensor.matmul(rep[:, :], i4_sb[:, :], eT, start=True, stop=True)
            for kt in range(KT):
                c0 = 32 * (KT * bh + kt)
                nc.vector.tensor_tensor(
                    a2_sb[:, c0 : c0 + 32],
                    rep[:, :],
                    mc_sb[:, 32 * kt : 32 * kt + 32],
                    mybir.AluOpType.mult,
                )

        for bh in range(2):
            ops_t = po.tile([32, DQK], F32, tag="out")
            for kt in range(KT):
                vt = vp.tile([128, DQK], BF16, tag="v")
                nc.sync.dma_start(vt[:, :], v_d[bh, 128 * kt : 128 * (kt + 1), :])
                c0 = 32 * (KT * bh + kt)
                a2c = a2_sb[:, c0 : c0 + 32]
                for n in range(NB):
                    nc.tensor.matmul(
                        ops_t[:, 512 * n : 512 * (n + 1)],
                        a2c,
                        vt[:, 512 * n : 512 * (n + 1)],
                        start=(kt == 0),
                        stop=(kt == KT - 1),
                    )
            ot = outp.tile([32, DQK], F32, tag="ot")
            for n in range(NB):
                nc.scalar.mul(
                    ot[:, 512 * n : 512 * (n + 1)],
                    ops_t[:, 512 * n : 512 * (n + 1)],
                    ri_sb[:, bh : bh + 1],
                )
            nc.sync.dma_start(out_d[bh], ot[:, :])

    nc.finalize()
    return nc


_PROG = None


def _get_program():
    global _PROG
    if _PROG is None:
        _PROG = _build_program()
    return _PROG


def _consts():
    mc = np.zeros((128, KT * 32), np.float32)
    for p in range(128):
        ii = p // 32
        for kt in range(KT):
            mc[p, 32 * kt + 4 * kt + ii] = 1.0
    i4t = np.tile(np.eye(32, dtype=np.float32), (1, 4))
    return mc, i4t


def make_in_maps(Q, K, V, mask):
    Q = np.asarray(Q)
    K = np.asarray(K)
    V = np.asarray(V)
    mask = np.asarray(mask)
    mc, i4t = _consts()
    in_maps = []
    for c in range(NCORES):
        pairs = [(2 * c) // H, (2 * c) % H], [(2 * c + 1) // H, (2 * c + 1) % H]
        cols = [Q[b, h].T for b, h in pairs] + [K[b, h].T for b, h in pairs]
        stack = np.concatenate(cols, axis=1)  # [3072, 128]
        qkt = (
            np.ascontiguousarray(stack.reshape(NCH, 128, 128).transpose(1, 0, 2))
            .reshape(128, NCH * 128)
            .astype(ml_dtypes.bfloat16)
        )
        mb = np.concatenate(
            [
                np.where(mask[b, h] == 0, np.float32(NEG), np.float32(0.0))
                for b, h in pairs
            ],
            axis=1,
        ).astype(np.float32)
        v2 = np.stack(
            [
                np.ascontiguousarray(V[b, h].transpose(1, 0, 2, 3)).reshape(
                    KT * 128, DQK
                )
                for b, h in pairs
            ]
        ).astype(ml_dtypes.bfloat16)
        in_maps.append({"qkt": qkt, "mb": mb, "v": v2, "mconst": mc, "i4t": i4t})
    return in_maps


def kernel(Q=None, K=None, V=None, mask=None, _trace=False, **_ignored):
    in_maps = make_in_maps(Q, K, V, mask)
    nc = _get_program()
    res = run_bass_kernel_spmd(nc, in_maps, list(range(NCORES)), trace=_trace)
    outs = np.stack([r["out"] for r in res.results])  # [8, 2, 32, 3072]
    out = outs.reshape(B, H, N, T, D)
    if _trace:
        return out, res
    return out


# revision 22
# speedup vs baseline: 1.8125x; 1.0377x over previous
import sys

if "/opt/trn_rl_repo" not in sys.path:
    sys.path.insert(0, "/opt/trn_rl_repo")

from contextlib import ExitStack

import ml_dtypes
import numpy as np

import concourse.bacc as bacc
import concourse.bass as bass
import concourse.mybir as mybir
import concourse.tile as tile
from concourse.bass_utils import run_bass_kernel_spmd

B, H, N, T, D = 4, 4, 32, 96, 32
DQK = T * D  # 3072
SCALE = float(DQK**0.5)
NCORES = 8
NCH = DQK // 128  # 24 contraction chunks for Q.K
KT = 8  # V row tiles per (b,h): 1024 rows / 128
NB = DQK // 512  # 6 psum column chunks
F32 = mybir.dt.float32
F32R = mybir.dt.float32r
BF16 = mybir.dt.bfloat16
NEG = -1.0e30


def _build_program():
    nc = bacc.Bacc()
    qkt_d = nc.declare_dram_parameter("qkt", [128, NCH * 128], BF16, isOutput=False)
    mb_d = nc.declare_dram_parameter("mb", [32, 64], F32, isOutput=False)
    v_d = nc.declare_dram_parameter("v", [2, KT * 128, DQK], BF16, isOutput=False)
    mc_d = nc.declare_dram_parameter("mconst", [128, KT * 32], F32, isOutput=False)
    i4_d = nc.declare_dram_parameter("i4t", [32, 128], F32, isOutput=False)
    out_d = nc.declare_dram_parameter("out", [2, 32, DQK], F32, isOutput=True)

    with tile.TileContext(nc) as tc, ExitStack() as ctx:
        sb = ctx.enter_context(tc.tile_pool(name="sb", bufs=1))
        vp = ctx.enter_context(tc.tile_pool(name="vp", bufs=1))
        outp = ctx.enter_context(tc.tile_pool(name="outp", bufs=2))
        pg = ctx.enter_context(tc.tile_pool(name="pg", bufs=1, space="PSUM"))
        pr = ctx.enter_context(tc.tile_pool(name="pr", bufs=1, space="PSUM"))
        po = ctx.enter_context(tc.tile_pool(name="po", bufs=1, space="PSUM"))

        qkt_sb = sb.tile([128, NCH * 128], BF16, tag="qkt")
        mb_sb = sb.tile([32, 64], F32, tag="mb")
        mc_sb = sb.tile([128, KT * 32], F32, tag="mc")
        i4_sb = sb.tile([32, 128], F32, tag="i4")
        a2_sb = sb.tile([128, 2 * KT * 32], BF16, tag="a2")
        t_sb = sb.tile([32, 64], F32, tag="t")
        e_sb = sb.tile([32, 64], F32, tag="e")
        en_sb = sb.tile([32, 64], F32, tag="en")
        eT_sb = sb.tile([32, 64], F32, tag="eT")
        nm_sb = sb.tile([32, 2], F32, tag="nm")
        nms_sb = sb.tile([32, 2], F32, tag="nms")
        rs_sb = sb.tile([32, 2], F32, tag="rs")
        ri_sb = sb.tile([32, 2], F32, tag="ri")

        nc.sync.dma_start(qkt_sb[:, :], qkt_d[:, :])
        nc.gpsimd.dma_start(mc_sb[:, :], mc_d[:, :])
        nc.gpsimd.dma_start(i4_sb[:, :], i4_d[:, :])
        nc.gpsimd.dma_start(mb_sb[:, :], mb_d[:, :])

        # Prefetch all V tiles up front so the sync ring streams them
        # back-to-back, never blocked behind output descriptors.
        vts = []
        for bh in range(2):
            row = []
            for kt in range(KT):
                vt = vp.tile([128, DQK], BF16, tag=f"v{bh}_{kt}")
                nc.sync.dma_start(
                    vt[:, :], v_d[bh, 128 * kt : 128 * (kt + 1), :]
                )
                row.append(vt)
            vts.append(row)

        # Gram quadrant Q.K of the stacked [Q0 Q1 K0 K1] columns: [64,64]
        # PSUM accumulator over 24 contraction chunks of 128.
        gram = pg.tile([64, 64], F32, tag="gram")
        for c in range(NCH):
            sl = qkt_sb[:, 128 * c : 128 * (c + 1)]
            nc.tensor.matmul(
                gram[:, :],
                sl[:, 0:64],
                sl[:, 64:128],
                start=(c == 0),
                stop=(c == NCH - 1),
            )

        for bh in range(2):
            blk = gram[32 * bh : 32 * bh + 32, 32 * bh : 32 * bh + 32]
            tcur = t_sb[:, 32 * bh : 32 * bh + 32]
            nc.vector.tensor_tensor(
                tcur, blk, mb_sb[:, 32 * bh : 32 * bh + 32], mybir.AluOpType.add
            )
            nm = nm_sb[:, bh : bh + 1]
            nc.vector.reduce_max(nm, tcur, axis=mybir.AxisListType.X, negate=True)
            nms = nms_sb[:, bh : bh + 1]
            nc.vector.tensor_scalar_mul(nms, nm, 1.0 / SCALE)
            ecur = e_sb[:, 32 * bh : 32 * bh + 32]
            rs = rs_sb[:, bh : bh + 1]
            nc.scalar.activation(
                ecur,
                tcur,
                mybir.ActivationFunctionType.Exp,
                bias=nms,
                scale=1.0 / SCALE,
                accum_out=rs,
            )
            nc.vector.reciprocal(ri_sb[:, bh : bh + 1], rs)
            encur = en_sb[:, 32 * bh : 32 * bh + 32]
            nc.vector.tensor_scalar_mul(encur, ecur, ri_sb[:, bh : bh + 1])
            eT = eT_sb[:, 32 * bh : 32 * bh + 32]
            nc.vector.transpose(eT, encur)
            rep = pr.tile([128, 32], F32, tag="rep")
            nc.tensor.matmul(rep[:, :], i4_sb[:, :], eT, start=True, stop=True)
            for kt in range(KT):
                c0 = 32 * (KT * bh + kt)
                nc.vector.tensor_tensor(
                    a2_sb[:, c0 : c0 + 32],
                    rep[:, :],
                    mc_sb[:, 32 * kt : 32 * kt + 32],
                    mybir.AluOpType.mult,
                )

        for bh in range(2):
            ops_t = po.tile([32, DQK], F32, tag="out")
            for kt in range(KT):
                vt = vts[bh][kt]
                c0 = 32 * (KT * bh + kt)
                a2c = a2_sb[:, c0 : c0 + 32]
                for n in range(NB):
                    nc.tensor.matmul(
                        ops_t[:, 512 * n : 512 * (n + 1)],
                        a2c,
                        vt[:, 512 * n : 512 * (n + 1)],
                        start=(kt == 0),
                        stop=(kt == KT - 1),
                    )
            ot = outp.tile([32, DQK], F32, tag="ot")
            for n in range(NB):
                src = ops_t[:, 512 * n : 512 * (n + 1)]
                dst = ot[:, 512 * n : 512 * (n + 1)]
                if n % 2 == 0:
                    nc.scalar.copy(dst, src)
                else:
                    nc.vector.tensor_scalar_mul(dst, src, 1.0)
            nc.sync.dma_start(out_d[bh], ot[:, :])

    nc.finalize()
    return nc


_PROG = None


def _get_program():
    global _PROG
    if _PROG is None:
        _PROG = _build_program()
    return _PROG


def _consts():
    mc = np.zeros((128, KT * 32), np.float32)
    for p in range(128):
        ii = p // 32
        for kt in range(KT):
            mc[p, 32 * kt + 4 * kt + ii] = 1.0
    i4t = np.tile(np.eye(32, dtype=np.float32), (1, 4))
    return mc, i4t


def make_in_maps(Q, K, V, mask):
    Q = np.asarray(Q)
    K = np.asarray(K)
    V = np.asarray(V)
    mask = np.asarray(mask)
    mc, i4t = _consts()
    in_maps = []
    for c in range(NCORES):
        pairs = [(2 * c) // H, (2 * c) % H], [(2 * c + 1) // H, (2 * c + 1) % H]
        cols = [Q[b, h].T for b, h in pairs] + [K[b, h].T for b, h in pairs]
        stack = np.concatenate(cols, axis=1)  # [3072, 128]
        qkt = (
            np.ascontiguousarray(stack.reshape(NCH, 128, 128).transpose(1, 0, 2))
            .reshape(128, NCH * 128)
            .astype(ml_dtypes.bfloat16)
        )
        mb = np.concatenate(
            [
                np.where(mask[b, h] == 0, np.float32(NEG), np.float32(0.0))
                for b, h in pairs
            ],
            axis=1,
        ).astype(np.float32)
        v2 = np.stack(
            [
                np.ascontiguousarray(V[b, h].transpose(1, 0, 2, 3)).reshape(
                    KT * 128, DQK
                )
                for b, h in pairs
            ]
        ).astype(ml_dtypes.bfloat16)
        in_maps.append({"qkt": qkt, "mb": mb, "v": v2, "mconst": mc, "i4t": i4t})
    return in_maps


def kernel(Q=None, K=None, V=None, mask=None, _trace=False, **_ignored):
    in_maps = make_in_maps(Q, K, V, mask)
    nc = _get_program()
    res = run_bass_kernel_spmd(nc, in_maps, list(range(NCORES)), trace=_trace)
    outs = np.stack([r["out"] for r in res.results])  # [8, 2, 32, 3072]
    out = outs.reshape(B, H, N, T, D)
    if _trace:
        return out, res
    return out


# revision 25
# speedup vs baseline: 1.8410x; 1.0157x over previous
import sys

if "/opt/trn_rl_repo" not in sys.path:
    sys.path.insert(0, "/opt/trn_rl_repo")

from contextlib import ExitStack

import ml_dtypes
import numpy as np

import concourse.bacc as bacc
import concourse.bass as bass
import concourse.mybir as mybir
import concourse.tile as tile
from concourse.bass_utils import run_bass_kernel_spmd

B, H, N, T, D = 4, 4, 32, 96, 32
DQK = T * D  # 3072
SCALE = float(DQK**0.5)
NCORES = 8
NCH = DQK // 128  # 24 contraction chunks for Q.K
KT = 8  # V row tiles per (b,h): 1024 rows / 128
NB = DQK // 512  # 6 psum column chunks
F32 = mybir.dt.float32
F32R = mybir.dt.float32r
BF16 = mybir.dt.bfloat16
NEG = -1.0e30


def _build_program():
    nc = bacc.Bacc()
    qkt_d = nc.declare_dram_parameter("qkt", [128, NCH * 128], BF16, isOutput=False)
    mb_d = nc.declare_dram_parameter("mb", [32, 64], F32, isOutput=False)
    v_d = nc.declare_dram_parameter("v", [2, KT * 128, DQK], BF16, isOutput=False)
    mc_d = nc.declare_dram_parameter("mconst", [128, KT * 32], F32, isOutput=False)
    i4_d = nc.declare_dram_parameter("i4t", [32, 128], F32, isOutput=False)
    out_d = nc.declare_dram_parameter("out", [2, 32, DQK], F32, isOutput=True)

    with tile.TileContext(nc) as tc, ExitStack() as ctx:
        sb = ctx.enter_context(tc.tile_pool(name="sb", bufs=1))
        vp = ctx.enter_context(tc.tile_pool(name="vp", bufs=1))
        outp = ctx.enter_context(tc.tile_pool(name="outp", bufs=2))
        pg = ctx.enter_context(tc.tile_pool(name="pg", bufs=1, space="PSUM"))
        pr = ctx.enter_context(tc.tile_pool(name="pr", bufs=1, space="PSUM"))
        po = ctx.enter_context(tc.tile_pool(name="po", bufs=1, space="PSUM"))

        qkt_sb = sb.tile([128, NCH * 128], BF16, tag="qkt")
        mb_sb = sb.tile([32, 64], F32, tag="mb")
        mc_sb = sb.tile([128, KT * 32], F32, tag="mc")
        i4_sb = sb.tile([32, 128], F32, tag="i4")
        a2_sb = sb.tile([128, 2 * KT * 32], BF16, tag="a2")
        t_sb = sb.tile([32, 64], F32, tag="t")
        e_sb = sb.tile([32, 64], F32, tag="e")
        en_sb = sb.tile([32, 64], F32, tag="en")
        eT_sb = sb.tile([32, 64], F32, tag="eT")
        nm_sb = sb.tile([32, 2], F32, tag="nm")
        nms_sb = sb.tile([32, 2], F32, tag="nms")
        rs_sb = sb.tile([32, 2], F32, tag="rs")
        ri_sb = sb.tile([32, 2], F32, tag="ri")

        for q in range(4):
            nc.sync.dma_start(
                qkt_sb[:, 768 * q : 768 * (q + 1)],
                qkt_d[:, 768 * q : 768 * (q + 1)],
            )
        nc.gpsimd.dma_start(mc_sb[:, :], mc_d[:, :])
        nc.gpsimd.dma_start(i4_sb[:, :], i4_d[:, :])
        nc.gpsimd.dma_start(mb_sb[:, :], mb_d[:, :])

        # Prefetch all V tiles up front so the sync ring streams them
        # back-to-back, never blocked behind output descriptors.
        vts = []
        for bh in range(2):
            row = []
            for kt in range(KT):
                vt = vp.tile([128, DQK], BF16, tag=f"v{bh}_{kt}")
                nc.sync.dma_start(
                    vt[:, :], v_d[bh, 128 * kt : 128 * (kt + 1), :]
                )
                row.append(vt)
            vts.append(row)

        # Gram quadrant Q.K of the stacked [Q0 Q1 K0 K1] columns: [64,64]
        # PSUM accumulator over 24 contraction chunks of 128.
        gram = pg.tile([64, 64], F32, tag="gram")
        for c in range(NCH):
            sl = qkt_sb[:, 128 * c : 128 * (c + 1)]
            nc.tensor.matmul(
                gram[:, :],
                sl[:, 0:64],
                sl[:, 64:128],
                start=(c == 0),
                stop=(c == NCH - 1),
            )

        for bh in range(2):
            blk = gram[32 * bh : 32 * bh + 32, 32 * bh : 32 * bh + 32]
            tcur = t_sb[:, 32 * bh : 32 * bh + 32]
            nc.vector.tensor_tensor(
                tcur, blk, mb_sb[:, 32 * bh : 32 * bh + 32], mybir.AluOpType.add
            )
            nm = nm_sb[:, bh : bh + 1]
            nc.vector.reduce_max(nm, tcur, axis=mybir.AxisListType.X, negate=True)
            nms = nms_sb[:, bh : bh + 1]
            nc.vector.tensor_scalar_mul(nms, nm, 1.0 / SCALE)
            ecur = e_sb[:, 32 * bh : 32 * bh + 32]
            rs = rs_sb[:, bh : bh + 1]
            nc.scalar.activation(
                ecur,
                tcur,
                mybir.ActivationFunctionType.Exp,
                bias=nms,
                scale=1.0 / SCALE,
                accum_out=rs,
            )
            nc.vector.reciprocal(ri_sb[:, bh : bh + 1], rs)
            encur = en_sb[:, 32 * bh : 32 * bh + 32]
            nc.vector.tensor_scalar_mul(encur, ecur, ri_sb[:, bh : bh + 1])
            eT = eT_sb[:, 32 * bh : 32 * bh + 32]
            nc.vector.transpose(eT, encur)
            rep = pr.tile([128, 32], F32, tag="rep")
            nc.tensor.matmul(rep[:, :], i4_sb[:, :], eT, start=True, stop=True)
            for kt in range(KT):
                c0 = 32 * (KT * bh + kt)
                nc.vector.tensor_tensor(
                    a2_sb[:, c0 : c0 + 32],
                    rep[:, :],
                    mc_sb[:, 32 * kt : 32 * kt + 32],
                    mybir.AluOpType.mult,
                )

        for bh in range(2):
            opst = [
                po.tile([32, 512], F32, tag=f"o{n}", name=f"o{n}")
                for n in range(NB)
            ]
            for kt in range(KT):
                vt = vts[bh][kt]
                c0 = 32 * (KT * bh + kt)
                a2c = a2_sb[:, c0 : c0 + 32]
                for n in range(NB):
                    nc.tensor.matmul(
                        opst[n][:, :],
                        a2c,
                        vt[:, 512 * n : 512 * (n + 1)],
                        start=(kt == 0),
                        stop=(kt == KT - 1),
                    )
            ot = outp.tile([32, DQK], F32, tag="ot")
            for n in range(NB):
                dst = ot[:, 512 * n : 512 * (n + 1)]
                if n % 2 == 0:
                    nc.scalar.copy(dst, opst[n][:, :])
                else:
                    nc.vector.tensor_scalar_mul(dst, opst[n][:, :], 1.0)
                nc.sync.dma_start(out_d[bh][:, 512 * n : 512 * (n + 1)], dst)

    nc.finalize()
    return nc


_PROG = None


def _get_program():
    global _PROG
    if _PROG is None:
        _PROG = _build_program()
    return _PROG


def _consts():
    mc = np.zeros((128, KT * 32), np.float32)
    for p in range(128):
        ii = p // 32
        for kt in range(KT):
            mc[p, 32 * kt + 4 * kt + ii] = 1.0
    i4t = np.tile(np.eye(32, dtype=np.float32), (1, 4))
    return mc, i4t


def make_in_maps(Q, K, V, mask):
    Q = np.asarray(Q)
    K = np.asarray(K)
    V = np.asarray(V)
    mask = np.asarray(mask)
    mc, i4t = _consts()
    in_maps = []
    for c in range(NCORES):
        pairs = [(2 * c) // H, (2 * c) % H], [(2 * c + 1) // H, (2 * c + 1) % H]
        cols = [Q[b, h].T for b, h in pairs] + [K[b, h].T for b, h in pairs]
        stack = np.concatenate(cols, axis=1)  # [3072, 128]
        qkt = (
            np.ascontiguousarray(stack.reshape(NCH, 128, 128).transpose(1, 0, 2))
            .reshape(128, NCH * 128)
            .astype(ml_dtypes.bfloat16)
        )
        mb = np.concatenate(
            [
                np.where(mask[b, h] == 0, np.float32(NEG), np.float32(0.0))
                for b, h in pairs
            ],
            axis=1,
        ).astype(np.float32)
        v2 = np.stack(
            [
                np.ascontiguousarray(V[b, h].transpose(1, 0, 2, 3)).reshape(
                    KT * 128, DQK
                )
                for b, h in pairs
            ]
        ).astype(ml_dtypes.bfloat16)
        in_maps.append({"qkt": qkt, "mb": mb, "v": v2, "mconst": mc, "i4t": i4t})
    return in_maps


def kernel(Q=None, K=None, V=None, mask=None, _trace=False, **_ignored):
    in_maps = make_in_maps(Q, K, V, mask)
    nc = _get_program()
    res = run_bass_kernel_spmd(nc, in_maps, list(range(NCORES)), trace=_trace)
    outs = np.stack([r["out"] for r in res.results])  # [8, 2, 32, 3072]
    out = outs.reshape(B, H, N, T, D)
    if _trace:
        return out, res
    return out


# revision 27
# speedup vs baseline: 1.8811x; 1.0218x over previous
import sys

if "/opt/trn_rl_repo" not in sys.path:
    sys.path.insert(0, "/opt/trn_rl_repo")

from contextlib import ExitStack

import ml_dtypes
import numpy as np

import concourse.bacc as bacc
import concourse.bass as bass
import concourse.mybir as mybir
import concourse.tile as tile
from concourse.bass_utils import run_bass_kernel_spmd

B, H, N, T, D = 4, 4, 32, 96, 32
DQK = T * D  # 3072
SCALE = float(DQK**0.5)
NCORES = 8
NCH = DQK // 128  # 24 contraction chunks for Q.K
KT = 8  # V row tiles per (b,h): 1024 rows / 128
NB = DQK // 512  # 6 psum column chunks
F32 = mybir.dt.float32
F32R = mybir.dt.float32r
BF16 = mybir.dt.bfloat16
NEG = -1.0e30


def _build_program():
    nc = bacc.Bacc()
    qkt_d = nc.declare_dram_parameter("qkt", [128, NCH * 128], BF16, isOutput=False)
    mb_d = nc.declare_dram_parameter("mb", [32, 64], F32, isOutput=False)
    v_d = nc.declare_dram_parameter("v", [2, KT * 128, DQK], BF16, isOutput=False)
    mc_d = nc.declare_dram_parameter("mconst", [128, KT * 32], F32, isOutput=False)
    i4_d = nc.declare_dram_parameter("i4t", [32, 128], F32, isOutput=False)
    out_d = nc.declare_dram_parameter("out", [2, 32, DQK], F32, isOutput=True)

    with tile.TileContext(nc) as tc, ExitStack() as ctx:
        sb = ctx.enter_context(tc.tile_pool(name="sb", bufs=1))
        vp = ctx.enter_context(tc.tile_pool(name="vp", bufs=1))
        outp = ctx.enter_context(tc.tile_pool(name="outp", bufs=2))
        pg = ctx.enter_context(tc.tile_pool(name="pg", bufs=1, space="PSUM"))
        pr = ctx.enter_context(tc.tile_pool(name="pr", bufs=1, space="PSUM"))
        po = ctx.enter_context(tc.tile_pool(name="po", bufs=1, space="PSUM"))

        qkt_sb = sb.tile([128, NCH * 128], BF16, tag="qkt")
        mb_sb = sb.tile([32, 64], F32, tag="mb")
        mc_sb = sb.tile([128, KT * 32], F32, tag="mc")
        i4_sb = sb.tile([32, 128], F32, tag="i4")
        a2_sb = sb.tile([128, 2 * KT * 32], BF16, tag="a2")
        t_sb = sb.tile([32, 64], F32, tag="t")
        e_sb = sb.tile([32, 64], F32, tag="e")
        en_sb = sb.tile([32, 64], F32, tag="en")
        eT_sb = sb.tile([32, 64], F32, tag="eT")
        nm_sb = sb.tile([32, 2], F32, tag="nm")
        nms_sb = sb.tile([32, 2], F32, tag="nms")
        rs_sb = sb.tile([32, 2], F32, tag="rs")
        ri_sb = sb.tile([32, 2], F32, tag="ri")

        nc.scalar.dma_start(qkt_sb[:, :], qkt_d[:, :])
        nc.gpsimd.dma_start(mc_sb[:, :], mc_d[:, :])
        nc.gpsimd.dma_start(i4_sb[:, :], i4_d[:, :])
        nc.gpsimd.dma_start(mb_sb[:, :], mb_d[:, :])

        # Prefetch all V tiles up front so the sync ring streams them
        # back-to-back, never blocked behind output descriptors.
        vts = []
        for bh in range(2):
            row = []
            for kt in range(KT):
                vt = vp.tile([128, DQK], BF16, tag=f"v{bh}_{kt}")
                nc.sync.dma_start(
                    vt[:, :], v_d[bh, 128 * kt : 128 * (kt + 1), :]
                )
                row.append(vt)
            vts.append(row)

        # Gram quadrant Q.K of the stacked [Q0 Q1 K0 K1] columns: [64,64]
        # PSUM accumulator over 24 contraction chunks of 128.
        gram = pg.tile([64, 64], F32, tag="gram")
        for c in range(NCH):
            sl = qkt_sb[:, 128 * c : 128 * (c + 1)]
            nc.tensor.matmul(
                gram[:, :],
                sl[:, 0:64],
                sl[:, 64:128],
                start=(c == 0),
                stop=(c == NCH - 1),
            )

        for bh in range(2):
            blk = gram[32 * bh : 32 * bh + 32, 32 * bh : 32 * bh + 32]
            tcur = t_sb[:, 32 * bh : 32 * bh + 32]
            nc.vector.tensor_tensor(
                tcur, blk, mb_sb[:, 32 * bh : 32 * bh + 32], mybir.AluOpType.add
            )
            nm = nm_sb[:, bh : bh + 1]
            nc.vector.reduce_max(nm, tcur, axis=mybir.AxisListType.X, negate=True)
            nms = nms_sb[:, bh : bh + 1]
            nc.vector.tensor_scalar_mul(nms, nm, 1.0 / SCALE)
            ecur = e_sb[:, 32 * bh : 32 * bh + 32]
            rs = rs_sb[:, bh : bh + 1]
            nc.scalar.activation(
                ecur,
                tcur,
                mybir.ActivationFunctionType.Exp,
                bias=nms,
                scale=1.0 / SCALE,
                accum_out=rs,
            )
            nc.vector.reciprocal(ri_sb[:, bh : bh + 1], rs)
            encur = en_sb[:, 32 * bh : 32 * bh + 32]
            nc.vector.tensor_scalar_mul(encur, ecur, ri_sb[:, bh : bh + 1])
            eT = eT_sb[:, 32 * bh : 32 * bh + 32]
            nc.vector.transpose(eT, encur)
            rep = pr.tile([128, 32], F32, tag="rep")
            nc.tensor.matmul(rep[:, :], i4_sb[:, :], eT, start=True, stop=True)
            for kt in range(KT):
                c0 = 32 * (KT * bh + kt)
                nc.vector.tensor_tensor(
                    a2_sb[:, c0 : c0 + 32],
                    rep[:, :],
                    mc_sb[:, 32 * kt : 32 * kt + 32],
                    mybir.AluOpType.mult,
                )

        for bh in range(2):
            opst = [
                po.tile([32, 512], F32, tag=f"o{n}", name=f"o{n}")
                for n in range(NB)
            ]
            for kt in range(KT):
                vt = vts[bh][kt]
                c0 = 32 * (KT * bh + kt)
                a2c = a2_sb[:, c0 : c0 + 32]
                for n in range(NB):
                    nc.tensor.matmul(
                        opst[n][:, :],
                        a2c,
                        vt[:, 512 * n : 512 * (n + 1)],
                        start=(kt == 0),
                        stop=(kt == KT - 1),
                    )
            ot = outp.tile([32, DQK], F32, tag="ot")
            for n in range(NB):
                dst = ot[:, 512 * n : 512 * (n + 1)]
                if n % 2 == 0:
                    nc.scalar.copy(dst, opst[n][:, :])
                else:
                    nc.vector.tensor_scalar_mul(dst, opst[n][:, :], 1.0)
            nc.scalar.dma_start(out_d[bh], ot[:, :])

    nc.finalize()
    return nc


_PROG = None


def _get_program():
    global _PROG
    if _PROG is None:
        _PROG = _build_program()
    return _PROG


def _consts():
    mc = np.zeros((128, KT * 32), np.float32)
    for p in range(128):
        ii = p // 32
        for kt in range(KT):
            mc[p, 32 * kt + 4 * kt + ii] = 1.0
    i4t = np.tile(np.eye(32, dtype=np.float32), (1, 4))
    return mc, i4t


def make_in_maps(Q, K, V, mask):
    Q = np.asarray(Q)
    K = np.asarray(K)
    V = np.asarray(V)
    mask = np.asarray(mask)
    mc, i4t = _consts()
    in_maps = []
    for c in range(NCORES):
        pairs = [(2 * c) // H, (2 * c) % H], [(2 * c + 1) // H, (2 * c + 1) % H]
        cols = [Q[b, h].T for b, h in pairs] + [K[b, h].T for b, h in pairs]
        stack = np.concatenate(cols, axis=1)  # [3072, 128]
        qkt = (
            np.ascontiguousarray(stack.reshape(NCH, 128, 128).transpose(1, 0, 2))
            .reshape(128, NCH * 128)
            .astype(ml_dtypes.bfloat16)
        )
        mb = np.concatenate(
            [
                np.where(mask[b, h] == 0, np.float32(NEG), np.float32(0.0))
                for b, h in pairs
            ],
            axis=1,
        ).astype(np.float32)
        v2 = np.stack(
            [
                np.ascontiguousarray(V[b, h].transpose(1, 0, 2, 3)).reshape(
                    KT * 128, DQK
                )
                for b, h in pairs
            ]
        ).astype(ml_dtypes.bfloat16)
        in_maps.append({"qkt": qkt, "mb": mb, "v": v2, "mconst": mc, "i4t": i4t})
    return in_maps


def kernel(Q=None, K=None, V=None, mask=None, _trace=False, **_ignored):
    in_maps = make_in_maps(Q, K, V, mask)
    nc = _get_program()
    res = run_bass_kernel_spmd(nc, in_maps, list(range(NCORES)), trace=_trace)
    outs = np.stack([r["out"] for r in res.results])  # [8, 2, 32, 3072]
    out = outs.reshape(B, H, N, T, D)
    if _trace:
        return out, res
    return out
